# revision 1
# baseline (speedup 1.0000x reference)
"""Trainium2 Bass kernel for the Elman-RNN reference (nn_Baseline_78005196030499).

Architecture (per core, data-parallel over batch, B=128 -> 16 rows/core):
  1. Host: cast emb to fp16, pre-transpose weights, build t-major int32 token
     index tables.
  2. Device prep: indirect-DMA gather of embedding rows (fp16), PE transpose
     to put E on partitions, fp16 input-projection GEMM (fp32 PSUM) with
     Wb+Ub folded in via the ACT epilogue -> wx [128, T*64] fp32 resident in
     SBUF (layout col = t*64 + jb*16 + b).
  3. Device scan: 512 sequential steps; per step 16 fp16 matmuls
     (stationary UwT tiles [128,128], moving hT [128,16]) accumulating into
     4 PSUM banks (one per output j-block), drained by DVE add (psum + wx_t)
     and ACT tanh back to fp16 hT tiles.  j-major order lets the drains trail
     the PE stream so PE never stalls.
  4. Final hidden state written out in fp32; the tiny V-projection and the
     batch gather happen on host in fp32.
"""
import sys

sys.path.insert(0, "/opt/trn_rl_repo")

import numpy as np

import concourse.bass as bass
import concourse.tile as tile
from concourse import bacc, mybir
from concourse.masks import make_identity

# Problem shape (hardcoded per contract).
VOCAB, EMB, HID, OUT = 50257, 256, 512, 2
B, T = 128, 512
NCORES = 8
BL = B // NCORES          # batch rows per core = 16
NTOK = BL * T             # tokens per core = 8192
P = 128
NJ = HID // P             # 4 j-blocks
NK = HID // P             # 4 k-chunks
NE = EMB // P             # 2 e-chunks
CHUNK = 512               # tokens per prep chunk (4 gathers of 128)
NCHUNK = NTOK // CHUNK    # 16
TL = CHUNK // BL          # timesteps per chunk = 32

F16 = mybir.dt.float16
F32 = mybir.dt.float32
I32 = mybir.dt.int32

_CACHED = {}


def build_module():
    nc = bacc.Bacc("TRN2", target_bir_lowering=False, debug=False)

    emb_d = nc.dram_tensor("emb16", [VOCAB, EMB], F16, kind="ExternalInput")
    wwT_d = nc.dram_tensor("wwT16", [P, NE, HID], F16, kind="ExternalInput")
    uwT_d = nc.dram_tensor("uwT16", [P, NK, HID], F16, kind="ExternalInput")
    bias_d = nc.dram_tensor("bias32", [P, NJ], F32, kind="ExternalInput")
    idx_d = nc.dram_tensor("idx32", [P, NTOK // P], I32, kind="ExternalInput")
    out_d = nc.dram_tensor("ht_out", [P, NJ * BL], F32, kind="ExternalOutput")

    with tile.TileContext(nc) as tc:
        with (
            tc.tile_pool(name="const", bufs=1) as cpool,
            tc.tile_pool(name="wxpool", bufs=1) as wxpool,
            tc.tile_pool(name="gather", bufs=6) as gpool,
            tc.tile_pool(name="xeTp", bufs=2) as xpool,
            tc.tile_pool(name="scan_sb", bufs=8) as spool,
        ):
            # --- resident constants -------------------------------------
            uwT = cpool.tile([P, NK, HID], F16)
            nc.sync.dma_start(uwT[:], uwT_d[:])
            wwT = cpool.tile([P, NE, HID], F16)
            nc.sync.dma_start(wwT[:], wwT_d[:])
            bias = cpool.tile([P, NJ], F32)
            nc.sync.dma_start(bias[:], bias_d[:])
            idx = cpool.tile([P, NTOK // P], I32)
            nc.sync.dma_start(idx[:], idx_d[:])
            ident = cpool.tile([P, P], F16)
            make_identity(nc, ident[:])

            # wx buffer: [128, T * 64] fp32, col = t*64 + jb*16 + b
            wx = wxpool.tile([P, T * NJ * BL], F32)
            wx_view = wx[:].rearrange("p (t j b) -> p t j b", t=T, j=NJ, b=BL)

            # --- prep: gather + transpose + GEMM ------------------------
            with tc.tile_pool(name="prep_ps", bufs=2, space="PSUM") as ppool:
                for c in range(NCHUNK):
                    xeT = xpool.tile([P, NE, CHUNK], F16)
                    for g in range(CHUNK // P):
                        col = c * (CHUNK // P) + g
                        xe_g = gpool.tile([P, EMB], F16)
                        nc.gpsimd.indirect_dma_start(
                            out=xe_g[:],
                            out_offset=None,
                            in_=emb_d[:],
                            in_offset=bass.IndirectOffsetOnAxis(
                                ap=idx[:, col:col + 1], axis=0
                            ),
                        )
                        tp = ppool.tile([P, NE, P], F16)
                        for e in range(NE):
                            nc.tensor.transpose(
                                tp[:, e, :], xe_g[:, e * P:(e + 1) * P], ident[:]
                            )
                        nc.vector.tensor_copy(
                            xeT[:, :, g * P:(g + 1) * P], tp[:]
                        )
                    for jb in range(NJ):
                        wxps = ppool.tile([P, CHUNK], F32)
                        for e in range(NE):
                            nc.tensor.matmul(
                                wxps[:],
                                wwT[:, e, jb * P:(jb + 1) * P],
                                xeT[:, e, :],
                                start=(e == 0),
                                stop=(e == NE - 1),
                            )
                        nc.scalar.activation(
                            wx_view[:, c * TL:(c + 1) * TL, jb, :],
                            wxps[:],
                            mybir.ActivationFunctionType.Identity,
                            bias=bias[:, jb:jb + 1],
                            scale=1.0,
                        )

            # --- scan: 512 steps ---------------------------------------
            hT32 = cpool.tile([P, NJ * BL], F32)
            with tc.tile_pool(name="scan_ps", bufs=8, space="PSUM") as scps:
                h_cur = []
                for k in range(NK):
                    h0 = spool.tile([P, BL], F16, tag="h")
                    nc.gpsimd.memset(h0[:], 0.0)
                    h_cur.append(h0)
                for t in range(T):
                    h_nxt = []
                    for jb in range(NJ):
                        ps = scps.tile([P, BL], F32, tag="ps")
                        for k in range(NK):
                            nc.tensor.matmul(
                                ps[:],
                                uwT[:, k, jb * P:(jb + 1) * P],
                                h_cur[k][:],
                                start=(k == 0),
                                stop=(k == NK - 1),
                            )
                        a = spool.tile([P, BL], F32, tag="a")
                        nc.vector.tensor_add(a[:], ps[:], wx_view[:, t, jb, :])
                        if t < T - 1:
                            h = spool.tile([P, BL], F16, tag="h")
                            nc.scalar.activation(
                                h[:], a[:], mybir.ActivationFunctionType.Tanh
                            )
                            h_nxt.append(h)
                        else:
                            nc.scalar.activation(
                                hT32[:, jb * BL:(jb + 1) * BL],
                                a[:],
                                mybir.ActivationFunctionType.Tanh,
                            )
                    h_cur = h_nxt

            nc.sync.dma_start(out_d[:], hT32[:])

    nc.compile()
    return nc


def _get_module():
    if "nc" not in _CACHED:
        _CACHED["nc"] = build_module()
    return _CACHED["nc"]


def _host_inputs(x, emb, Ww, Wb, Uw, Ub):
    emb16 = emb.astype(np.float16)
    wwT16 = np.ascontiguousarray(
        Ww.T.reshape(NE, P, HID).transpose(1, 0, 2)
    ).astype(np.float16)
    uwT16 = np.ascontiguousarray(
        Uw.T.reshape(NK, P, HID).transpose(1, 0, 2)
    ).astype(np.float16)
    bias32 = np.ascontiguousarray(
        (Wb + Ub).astype(np.float32).reshape(NJ, P).T
    )
    in_maps = []
    for c in range(NCORES):
        xc = x[c * BL:(c + 1) * BL, :]              # [16, 512] int
        tok = np.ascontiguousarray(xc.T).reshape(-1)  # t-major [8192]
        idx32 = np.ascontiguousarray(
            tok.reshape(NTOK // P, P).T
        ).astype(np.int32)                           # [128, 64]
        in_maps.append({
            "emb16": emb16,
            "wwT16": wwT16,
            "uwT16": uwT16,
            "bias32": bias32,
            "idx32": idx32,
        })
    return in_maps


def _ht_to_h(ht):
    # ht [128, 64] f32, ht[p, kb*16+b] = h[b, kb*128+p]
    return np.ascontiguousarray(
        ht.reshape(P, NJ, BL).transpose(2, 1, 0).reshape(BL, HID)
    )


def run_device(x, emb, Ww, Wb, Uw, Ub, trace=False):
    from concourse.bass_utils import run_bass_kernel_spmd

    nc = _get_module()
    in_maps = _host_inputs(x, emb, Ww, Wb, Uw, Ub)
    res = run_bass_kernel_spmd(
        nc, in_maps, list(range(NCORES)), trace=trace,
        trace_cores=list(range(NCORES)) if trace else None,
    )
    hs = [_ht_to_h(res.results[c]["ht_out"]) for c in range(NCORES)]
    h_full = np.concatenate(hs, axis=0)              # [128, 512] f32
    return h_full, res


def kernel(x, emb, Ww, Wb, Uw, Ub, Vw, Vb):
    x = np.asarray(x)
    emb = np.asarray(emb, dtype=np.float32)
    Ww = np.asarray(Ww, dtype=np.float32)
    Wb = np.asarray(Wb, dtype=np.float32)
    Uw = np.asarray(Uw, dtype=np.float32)
    Ub = np.asarray(Ub, dtype=np.float32)
    Vw = np.asarray(Vw, dtype=np.float32)
    Vb = np.asarray(Vb, dtype=np.float32)

    h_full, _ = run_device(x, emb, Ww, Wb, Uw, Ub, trace=False)
    logits = h_full @ Vw.T.astype(np.float32) + Vb   # [128, 2]
    return logits.astype(np.float32)


# revision 2
# speedup vs baseline: 1.1353x; 1.1353x over previous
"""Trainium2 Bass kernel for the Elman-RNN reference (nn_Baseline_78005196030499).

Architecture (per core, data-parallel over batch, B=128 -> 16 rows/core):
  1. Host: cast emb to fp16, pre-transpose weights, build t-major int32 token
     index tables.
  2. Device prep: indirect-DMA gather of embedding rows (fp16), PE transpose
     to put E on partitions, fp16 input-projection GEMM (fp32 PSUM) with
     Wb+Ub folded in via the ACT epilogue -> wx [128, T*64] fp32 resident in
     SBUF (layout col = t*64 + jb*16 + b).
  3. Device scan: 512 sequential steps; per step 16 fp16 matmuls
     (stationary UwT tiles [128,128], moving hT [128,16]) accumulating into
     4 PSUM banks (one per output j-block), drained by DVE add (psum + wx_t)
     and ACT tanh back to fp16 hT tiles.  j-major order lets the drains trail
     the PE stream so PE never stalls.
  4. Final hidden state written out in fp32; the tiny V-projection and the
     batch gather happen on host in fp32.
"""
import sys

sys.path.insert(0, "/opt/trn_rl_repo")

import numpy as np

import concourse.bass as bass
import concourse.tile as tile
from concourse import bacc, mybir
from concourse.masks import make_identity

# Problem shape (hardcoded per contract).
VOCAB, EMB, HID, OUT = 50257, 256, 512, 2
B, T = 128, 512
NCORES = 8
BL = B // NCORES          # batch rows per core = 16
NTOK = BL * T             # tokens per core = 8192
P = 128
NJ = HID // P             # 4 j-blocks
NK = HID // P             # 4 k-chunks
NE = EMB // P             # 2 e-chunks
CHUNK = 512               # tokens per prep chunk (4 gathers of 128)
NCHUNK = NTOK // CHUNK    # 16
TL = CHUNK // BL          # timesteps per chunk = 32

F16 = mybir.dt.float16
F32 = mybir.dt.float32
I32 = mybir.dt.int32

_CACHED = {}


def build_module():
    nc = bacc.Bacc("TRN2", target_bir_lowering=False, debug=False)

    emb_d = nc.dram_tensor("emb16", [VOCAB, EMB], F16, kind="ExternalInput")
    wwT_d = nc.dram_tensor("wwT16", [P, NE, HID], F16, kind="ExternalInput")
    uwT_d = nc.dram_tensor("uwT16", [P, NK, HID], F16, kind="ExternalInput")
    bias_d = nc.dram_tensor("bias32", [P, NJ], F32, kind="ExternalInput")
    idx_d = nc.dram_tensor("idx32", [P, NTOK // P], I32, kind="ExternalInput")
    out_d = nc.dram_tensor("ht_out", [P, NJ * BL], F32, kind="ExternalOutput")

    with tile.TileContext(nc) as tc:
        with (
            tc.tile_pool(name="const", bufs=1) as cpool,
            tc.tile_pool(name="wxpool", bufs=1) as wxpool,
            tc.tile_pool(name="gather", bufs=6) as gpool,
            tc.tile_pool(name="xeTp", bufs=2) as xpool,
            tc.tile_pool(name="scan_sb", bufs=8) as spool,
        ):
            # --- resident constants -------------------------------------
            uwT = cpool.tile([P, NK, HID], F16)
            nc.sync.dma_start(uwT[:], uwT_d[:])
            wwT = cpool.tile([P, NE, HID], F16)
            nc.sync.dma_start(wwT[:], wwT_d[:])
            bias = cpool.tile([P, NJ], F32)
            nc.sync.dma_start(bias[:], bias_d[:])
            idx = cpool.tile([P, NTOK // P], I32)
            nc.sync.dma_start(idx[:], idx_d[:])
            ident = cpool.tile([P, P], F16)
            make_identity(nc, ident[:])

            # wx buffer: [128, T * 64] fp32, col = t*64 + jb*16 + b
            wx = wxpool.tile([P, T * NJ * BL], F32)
            wx_view = wx[:].rearrange("p (t j b) -> p t j b", t=T, j=NJ, b=BL)

            # --- prep: gather + transpose + GEMM ------------------------
            with tc.tile_pool(name="prep_ps", bufs=2, space="PSUM") as ppool:
                for c in range(NCHUNK):
                    xeT = xpool.tile([P, NE, CHUNK], F16)
                    for g in range(CHUNK // P):
                        col = c * (CHUNK // P) + g
                        xe_g = gpool.tile([P, EMB], F16)
                        nc.gpsimd.indirect_dma_start(
                            out=xe_g[:],
                            out_offset=None,
                            in_=emb_d[:],
                            in_offset=bass.IndirectOffsetOnAxis(
                                ap=idx[:, col:col + 1], axis=0
                            ),
                        )
                        tp = ppool.tile([P, NE, P], F16)
                        for e in range(NE):
                            nc.tensor.transpose(
                                tp[:, e, :], xe_g[:, e * P:(e + 1) * P], ident[:]
                            )
                        nc.vector.tensor_copy(
                            xeT[:, :, g * P:(g + 1) * P], tp[:]
                        )
                    for jb in range(NJ):
                        wxps = ppool.tile([P, CHUNK], F32)
                        for e in range(NE):
                            nc.tensor.matmul(
                                wxps[:],
                                wwT[:, e, jb * P:(jb + 1) * P],
                                xeT[:, e, :],
                                start=(e == 0),
                                stop=(e == NE - 1),
                            )
                        nc.scalar.activation(
                            wx_view[:, c * TL:(c + 1) * TL, jb, :],
                            wxps[:],
                            mybir.ActivationFunctionType.Identity,
                            bias=bias[:, jb:jb + 1],
                            scale=1.0,
                        )

            # --- scan: 512 steps ---------------------------------------
            # Two half-steps per step: j-blocks {0,1} then {2,3}.  Each half
            # accumulates 8 matmuls into one PSUM tile [128, 32], then DVE
            # adds wx in place (psum += wx) and ACT tanh reads PSUM directly
            # into an fp16 h tile [128, 32].  Halves alternate banks so the
            # drains of one half overlap the matmuls of the other.
            hT32 = cpool.tile([P, NJ * BL], F32)
            wx_half = wx[:].rearrange(
                "p (t half c) -> p t half c", t=T, half=2, c=2 * BL
            )
            with tc.tile_pool(name="scan_ps", bufs=4, space="PSUM") as scps:
                h_cur = []
                for kp in range(NK // 2):
                    h0 = spool.tile([P, 2 * BL], F16, tag="h")
                    nc.gpsimd.memset(h0[:], 0.0)
                    h_cur.append(h0)
                for t in range(T):
                    h_nxt = []
                    for half in range(2):
                        ps = scps.tile([P, 2 * BL], F32, tag="ps")
                        for jj in range(2):
                            jb = half * 2 + jj
                            for k in range(NK):
                                nc.tensor.matmul(
                                    ps[:, jj * BL:(jj + 1) * BL],
                                    uwT[:, k, jb * P:(jb + 1) * P],
                                    h_cur[k // 2][:, (k % 2) * BL:(k % 2 + 1) * BL],
                                    start=(k == 0),
                                    stop=(k == NK - 1),
                                )
                        nc.vector.tensor_add(
                            ps[:], ps[:], wx_half[:, t, half, :]
                        )
                        if t < T - 1:
                            h = spool.tile([P, 2 * BL], F16, tag="h")
                            nc.scalar.activation(
                                h[:], ps[:], mybir.ActivationFunctionType.Tanh
                            )
                            h_nxt.append(h)
                        else:
                            nc.scalar.activation(
                                hT32[:, half * 2 * BL:(half + 1) * 2 * BL],
                                ps[:],
                                mybir.ActivationFunctionType.Tanh,
                            )
                    h_cur = h_nxt

            nc.sync.dma_start(out_d[:], hT32[:])

    nc.compile()
    return nc


def _get_module():
    if "nc" not in _CACHED:
        _CACHED["nc"] = build_module()
    return _CACHED["nc"]


def _host_inputs(x, emb, Ww, Wb, Uw, Ub):
    emb16 = emb.astype(np.float16)
    wwT16 = np.ascontiguousarray(
        Ww.T.reshape(NE, P, HID).transpose(1, 0, 2)
    ).astype(np.float16)
    uwT16 = np.ascontiguousarray(
        Uw.T.reshape(NK, P, HID).transpose(1, 0, 2)
    ).astype(np.float16)
    bias32 = np.ascontiguousarray(
        (Wb + Ub).astype(np.float32).reshape(NJ, P).T
    )
    in_maps = []
    for c in range(NCORES):
        xc = x[c * BL:(c + 1) * BL, :]              # [16, 512] int
        tok = np.ascontiguousarray(xc.T).reshape(-1)  # t-major [8192]
        idx32 = np.ascontiguousarray(
            tok.reshape(NTOK // P, P).T
        ).astype(np.int32)                           # [128, 64]
        in_maps.append({
            "emb16": emb16,
            "wwT16": wwT16,
            "uwT16": uwT16,
            "bias32": bias32,
            "idx32": idx32,
        })
    return in_maps


def _ht_to_h(ht):
    # ht [128, 64] f32, ht[p, kb*16+b] = h[b, kb*128+p]
    return np.ascontiguousarray(
        ht.reshape(P, NJ, BL).transpose(2, 1, 0).reshape(BL, HID)
    )


def run_device(x, emb, Ww, Wb, Uw, Ub, trace=False):
    from concourse.bass_utils import run_bass_kernel_spmd

    nc = _get_module()
    in_maps = _host_inputs(x, emb, Ww, Wb, Uw, Ub)
    res = run_bass_kernel_spmd(
        nc, in_maps, list(range(NCORES)), trace=trace,
        trace_cores=list(range(NCORES)) if trace else None,
    )
    hs = [_ht_to_h(res.results[c]["ht_out"]) for c in range(NCORES)]
    h_full = np.concatenate(hs, axis=0)              # [128, 512] f32
    return h_full, res


def kernel(x, emb, Ww, Wb, Uw, Ub, Vw, Vb):
    x = np.asarray(x)
    emb = np.asarray(emb, dtype=np.float32)
    Ww = np.asarray(Ww, dtype=np.float32)
    Wb = np.asarray(Wb, dtype=np.float32)
    Uw = np.asarray(Uw, dtype=np.float32)
    Ub = np.asarray(Ub, dtype=np.float32)
    Vw = np.asarray(Vw, dtype=np.float32)
    Vb = np.asarray(Vb, dtype=np.float32)

    h_full, _ = run_device(x, emb, Ww, Wb, Uw, Ub, trace=False)
    logits = h_full @ Vw.T.astype(np.float32) + Vb   # [128, 2]
    return logits.astype(np.float32)


# revision 6
# speedup vs baseline: 1.4534x; 1.2801x over previous
"""Trainium2 Bass kernel for the Elman-RNN reference (nn_Baseline_78005196030499).

Architecture (per core, data-parallel over batch, B=128 -> 16 rows/core):
  1. Host: cast emb to fp16, pre-transpose weights, build t-major int32 token
     index tables.
  2. Device prep: indirect-DMA gather of embedding rows (fp16), PE transpose
     to put E on partitions, fp16 input-projection GEMM (fp32 PSUM) with
     Wb+Ub folded in via the ACT epilogue -> wx [128, T*64] fp32 resident in
     SBUF (layout col = t*64 + jb*16 + b).
  3. Device scan: 512 sequential steps; per step 16 fp16 matmuls
     (stationary UwT tiles [128,128], moving hT [128,16]) accumulating into
     4 PSUM banks (one per output j-block), drained by DVE add (psum + wx_t)
     and ACT tanh back to fp16 hT tiles.  j-major order lets the drains trail
     the PE stream so PE never stalls.
  4. Final hidden state written out in fp32; the tiny V-projection and the
     batch gather happen on host in fp32.
"""
import sys

sys.path.insert(0, "/opt/trn_rl_repo")

import numpy as np

import concourse.bass as bass
import concourse.tile as tile
from concourse import bacc, mybir
from concourse.masks import make_identity

# Problem shape (hardcoded per contract).
VOCAB, EMB, HID, OUT = 50257, 256, 512, 2
B, T = 128, 512
NCORES = 8
BL = B // NCORES          # batch rows per core = 16
NTOK = BL * T             # tokens per core = 8192
P = 128
NJ = HID // P             # 4 j-blocks
NK = HID // P             # 4 k-chunks
NE = EMB // P             # 2 e-chunks
CHUNK = 512               # tokens per prep chunk (4 gathers of 128)
NCHUNK = NTOK // CHUNK    # 16
TL = CHUNK // BL          # timesteps per chunk = 32

F16 = mybir.dt.float16
F32 = mybir.dt.float32
I32 = mybir.dt.int32

_CACHED = {}


def build_module():
    nc = bacc.Bacc("TRN2", target_bir_lowering=False, debug=False)

    emb_d = nc.dram_tensor("emb16", [VOCAB, EMB], F16, kind="ExternalInput")
    wwT_d = nc.dram_tensor("wwT16", [P, NE, HID], F16, kind="ExternalInput")
    uwT_d = nc.dram_tensor("uwT16", [P, NK, HID], F16, kind="ExternalInput")
    bias_d = nc.dram_tensor("bias32", [P, NJ], F32, kind="ExternalInput")
    idx_d = nc.dram_tensor("idx32", [P, NTOK // P], I32, kind="ExternalInput")
    out_d = nc.dram_tensor("ht_out", [P, NJ * BL], F32, kind="ExternalOutput")

    with tile.TileContext(nc) as tc:
        with (
            tc.tile_pool(name="const", bufs=1) as cpool,
            tc.tile_pool(name="wxpool", bufs=1) as wxpool,
            tc.tile_pool(name="gather", bufs=6) as gpool,
            tc.tile_pool(name="xeTp", bufs=2) as xpool,
            tc.tile_pool(name="scan_sb", bufs=8) as spool,
        ):
            # --- resident constants -------------------------------------
            uwT = cpool.tile([P, NK, HID], F16)
            nc.sync.dma_start(uwT[:], uwT_d[:])
            wwT = cpool.tile([P, NE, HID], F16)
            nc.sync.dma_start(wwT[:], wwT_d[:])
            bias = cpool.tile([P, NJ], F32)
            nc.sync.dma_start(bias[:], bias_d[:])
            idx = cpool.tile([P, NTOK // P], I32)
            nc.sync.dma_start(idx[:], idx_d[:])
            ident = cpool.tile([P, P], F16)
            make_identity(nc, ident[:])

            # wx buffer: [128, T * 64] fp16, col = t*64 + jb*16 + b
            wx = wxpool.tile([P, T * NJ * BL], F16)
            wx_view = wx[:].rearrange("p (t j b) -> p t j b", t=T, j=NJ, b=BL)

            # --- prep: gather + transpose + GEMM ------------------------
            with tc.tile_pool(name="prep_ps", bufs=2, space="PSUM") as ppool:
                for c in range(NCHUNK):
                    xeT = xpool.tile([P, NE, CHUNK], F16)
                    for g in range(CHUNK // P):
                        col = c * (CHUNK // P) + g
                        xe_g = gpool.tile([P, EMB], F16)
                        nc.gpsimd.indirect_dma_start(
                            out=xe_g[:],
                            out_offset=None,
                            in_=emb_d[:],
                            in_offset=bass.IndirectOffsetOnAxis(
                                ap=idx[:, col:col + 1], axis=0
                            ),
                        )
                        tp = ppool.tile([P, NE, P], F16)
                        for e in range(NE):
                            nc.tensor.transpose(
                                tp[:, e, :], xe_g[:, e * P:(e + 1) * P], ident[:]
                            )
                        nc.vector.tensor_copy(
                            xeT[:, :, g * P:(g + 1) * P], tp[:]
                        )
                    for jb in range(NJ):
                        wxps = ppool.tile([P, CHUNK], F32)
                        for e in range(NE):
                            nc.tensor.matmul(
                                wxps[:],
                                wwT[:, e, jb * P:(jb + 1) * P],
                                xeT[:, e, :],
                                start=(e == 0),
                                stop=(e == NE - 1),
                            )
                        nc.scalar.activation(
                            wx_view[:, c * TL:(c + 1) * TL, jb, :],
                            wxps[:],
                            mybir.ActivationFunctionType.Identity,
                            bias=bias[:, jb:jb + 1],
                            scale=1.0,
                        )

            # --- scan: 512 steps ---------------------------------------
            # Two PSUM tiles [128, 32] per step (j-blocks {0,1} / {2,3}).
            # wx is injected through the PE: an identity-weight matmul with
            # start=True opens each accumulation group with wx_t already in
            # PSUM (sets has_written), so the drain is a single ACT tanh
            # reading PSUM into an fp16 h tile.  The identity matmuls depend
            # only on static data, filling the PE bubble while it waits for
            # the previous step's tanh.
            hT32 = cpool.tile([P, NJ * BL], F32)
            wx_half = wx[:].rearrange(
                "p (t half c) -> p t half c", t=T, half=2, c=2 * BL
            )
            with tc.tile_pool(name="scan_ps", bufs=4, space="PSUM") as scps:
                h_cur = []
                for kp in range(NK // 2):
                    h0 = spool.tile([P, 2 * BL], F16, tag="h")
                    nc.gpsimd.memset(h0[:], 0.0)
                    h_cur.append(h0)
                for t in range(T):
                    pss = []
                    for half in range(2):
                        ps = scps.tile([P, 2 * BL], F32, tag="ps")
                        nc.tensor.matmul(
                            ps[:], ident[:], wx_half[:, t, half, :],
                            start=True, stop=False,
                        )
                        pss.append(ps)
                    h_nxt = []
                    for half in range(2):
                        ps = pss[half]
                        for jj in range(2):
                            jb = half * 2 + jj
                            for k in range(NK):
                                nc.tensor.matmul(
                                    ps[:, jj * BL:(jj + 1) * BL],
                                    uwT[:, k, jb * P:(jb + 1) * P],
                                    h_cur[k // 2][:, (k % 2) * BL:(k % 2 + 1) * BL],
                                    start=False,
                                    stop=(jj == 1 and k == NK - 1),
                                )
                        if t < T - 1:
                            h = spool.tile([P, 2 * BL], F16, tag="h")
                            nc.scalar.activation(
                                h[:], ps[:], mybir.ActivationFunctionType.Tanh
                            )
                            h_nxt.append(h)
                        else:
                            nc.scalar.activation(
                                hT32[:, half * 2 * BL:(half + 1) * 2 * BL],
                                ps[:],
                                mybir.ActivationFunctionType.Tanh,
                            )
                    h_cur = h_nxt

            nc.sync.dma_start(out_d[:], hT32[:])

    nc.compile()
    return nc


def _get_module():
    if "nc" not in _CACHED:
        _CACHED["nc"] = build_module()
    return _CACHED["nc"]


def _host_inputs(x, emb, Ww, Wb, Uw, Ub):
    emb16 = emb.astype(np.float16)
    wwT16 = np.ascontiguousarray(
        Ww.T.reshape(NE, P, HID).transpose(1, 0, 2)
    ).astype(np.float16)
    uwT16 = np.ascontiguousarray(
        Uw.T.reshape(NK, P, HID).transpose(1, 0, 2)
    ).astype(np.float16)
    bias32 = np.ascontiguousarray(
        (Wb + Ub).astype(np.float32).reshape(NJ, P).T
    )
    in_maps = []
    for c in range(NCORES):
        xc = x[c * BL:(c + 1) * BL, :]              # [16, 512] int
        tok = np.ascontiguousarray(xc.T).reshape(-1)  # t-major [8192]
        idx32 = np.ascontiguousarray(
            tok.reshape(NTOK // P, P).T
        ).astype(np.int32)                           # [128, 64]
        in_maps.append({
            "emb16": emb16,
            "wwT16": wwT16,
            "uwT16": uwT16,
            "bias32": bias32,
            "idx32": idx32,
        })
    return in_maps


def _ht_to_h(ht):
    # ht [128, 64] f32, ht[p, kb*16+b] = h[b, kb*128+p]
    return np.ascontiguousarray(
        ht.reshape(P, NJ, BL).transpose(2, 1, 0).reshape(BL, HID)
    )


def run_device(x, emb, Ww, Wb, Uw, Ub, trace=False):
    from concourse.bass_utils import run_bass_kernel_spmd

    nc = _get_module()
    in_maps = _host_inputs(x, emb, Ww, Wb, Uw, Ub)
    res = run_bass_kernel_spmd(
        nc, in_maps, list(range(NCORES)), trace=trace,
        trace_cores=list(range(NCORES)) if trace else None,
    )
    hs = [_ht_to_h(res.results[c]["ht_out"]) for c in range(NCORES)]
    h_full = np.concatenate(hs, axis=0)              # [128, 512] f32
    return h_full, res


def kernel(x, emb, Ww, Wb, Uw, Ub, Vw, Vb):
    x = np.asarray(x)
    emb = np.asarray(emb, dtype=np.float32)
    Ww = np.asarray(Ww, dtype=np.float32)
    Wb = np.asarray(Wb, dtype=np.float32)
    Uw = np.asarray(Uw, dtype=np.float32)
    Ub = np.asarray(Ub, dtype=np.float32)
    Vw = np.asarray(Vw, dtype=np.float32)
    Vb = np.asarray(Vb, dtype=np.float32)

    h_full, _ = run_device(x, emb, Ww, Wb, Uw, Ub, trace=False)
    logits = h_full @ Vw.T.astype(np.float32) + Vb   # [128, 2]
    return logits.astype(np.float32)
